# revision 9
# baseline (speedup 1.0000x reference)
"""Trainium2 Bass kernel for nn_ChiralEmbeddingModel (chiral tensor-product embedding).

Math (per atom n, with x = atomic_embeddings[n, 256:].reshape(128, 3)):
    ms   = mean(x^2)                       (over all 384 components)
    xh   = x / sqrt(ms + eps)              (rms_g folded into the weights)
    y    = w1' @ xh                        (w1'[u,v] = C1 * g[v] * w1[u,v])
    cr_i = eps_ijk xh_j y_k                (cross product per mul-channel)
    z    = w2' @ cr                        (w2'[u,v] = C2 * g[v] * w2[u,v])
    chi  = sum_i xh_i * z_i
    out  = chi @ Wo' + b                   (Wo'[u,o] = g[u] * W_out[o,u])

Device layout: atoms are tiled 512 at a time (4 chunks of 128).  The per-atom
rms scale is fused into the PE transpose (rhs = diag(s) instead of identity),
so everything downstream of the transpose is already normalized.  The cross
product subtractions are folded into the z matmuls via +/-w2' stationaries.
The bias is folded into the output PSUM via a rank-1 (ones x b) matmul.

Sharding: pure data-parallel over the atom axis across 8 NeuronCores; small
weights are replicated.  Host-side prep only slices/reshapes inputs and
transposes/scales the small replicated weight matrices.
"""

import numpy as np

N_TOTAL = 131072
N_CORES = 8
N_SHARD = N_TOTAL // N_CORES  # 16384
INV = 256
MUL = 128
EDIM = 3
F = MUL * EDIM  # 384
OUT = 512
EPS = 1e-6
C1 = (3.0 / 256.0) ** 0.5
C2 = (1.0 / 384.0) ** 0.5
P = 128
TILE_ATOMS = 512
NCHUNK = TILE_ATOMS // P  # 4

# which engine computes the sum-of-squares (scalar_tensor_tensor with accum):
#   "gpsimd" keeps the vector engine free (it is the bottleneck), falls back
#   to "vector" if the Q7 ucode path turns out not to support it.
MS_ENGINE = "vector"


def _build_nc(n_shard: int, ms_engine: str = MS_ENGINE):
    import concourse.bass as bass
    import concourse.bacc as bacc
    import concourse.tile as tile
    from concourse import mybir

    f32 = mybir.dt.float32
    Alu = mybir.AluOpType
    Act = mybir.ActivationFunctionType

    assert n_shard % TILE_ATOMS == 0
    n_tiles = n_shard // TILE_ATOMS

    nc = bacc.Bacc("TRN2", target_bir_lowering=False, debug=False)

    # Register EPS as a const AP (like the framework's 0.0/1.0) so activation
    # bias=EPS lowers to a dependency-free const read instead of a tracked AP.
    _eps_t = nc.alloc_sbuf_tensor("const-float32-eps", [128, 1], f32)
    nc.gpsimd.memset(_eps_t.ap(), EPS)
    nc.const_aps.aps[(f32, EPS)] = _eps_t.ap()
    nc.all_engine_barrier()

    xs = nc.dram_tensor("xs", [n_shard, F], f32, kind="ExternalInput").ap()
    w1t = nc.dram_tensor("w1t", [MUL, MUL], f32, kind="ExternalInput").ap()
    w2pt = nc.dram_tensor("w2pt", [MUL, MUL], f32, kind="ExternalInput").ap()
    w2mt = nc.dram_tensor("w2mt", [MUL, MUL], f32, kind="ExternalInput").ap()
    wot = nc.dram_tensor("wot", [MUL, OUT], f32, kind="ExternalInput").ap()
    ident = nc.dram_tensor("ident", [P, P], f32, kind="ExternalInput").ap()
    brow = nc.dram_tensor("brow", [1, OUT], f32, kind="ExternalInput").ap()
    onesrow = nc.dram_tensor("onesrow", [1, P], f32, kind="ExternalInput").ap()
    out = nc.dram_tensor("out", [n_shard, OUT], f32, kind="ExternalOutput").ap()

    # cross product index pairs: cr_0 = xh1*y2 - xh2*y1, etc.
    PLUS = [(1, 2), (2, 0), (0, 1)]
    MINUS = [(2, 1), (0, 2), (1, 0)]

    with tile.TileContext(nc) as tc:
        with (
            tc.tile_pool(name="singles", bufs=1) as singles,
            tc.tile_pool(name="xin", bufs=3) as xin_pool,
            tc.tile_pool(name="stats", bufs=2) as stats_pool,
            tc.tile_pool(name="sq", bufs=2) as sq_pool,
            tc.tile_pool(name="diag", bufs=8) as diag_pool,
            tc.tile_pool(name="xt", bufs=6) as xt_pool,
            tc.tile_pool(name="bprod", bufs=12) as b_pool,
            tc.tile_pool(name="cprod", bufs=6) as c_pool,
            tc.tile_pool(name="chi", bufs=4) as chi_pool,
            tc.tile_pool(name="outs", bufs=2) as out_pool,
            tc.tile_pool(name="psum", bufs=8, space="PSUM") as psum_pool,
        ):
            # ---- load replicated constants once ----
            w1t_sb = singles.tile([MUL, MUL], f32)
            w2pt_sb = singles.tile([MUL, MUL], f32)
            w2mt_sb = singles.tile([MUL, MUL], f32)
            wot_sb = singles.tile([MUL, OUT], f32)
            ident_sb = singles.tile([P, P], f32)
            brow_sb = singles.tile([1, OUT], f32)
            ones_sb = singles.tile([1, P], f32)
            nc.sync.dma_start(out=w1t_sb, in_=w1t)
            nc.sync.dma_start(out=w2pt_sb, in_=w2pt)
            nc.sync.dma_start(out=w2mt_sb, in_=w2mt)
            nc.sync.dma_start(out=wot_sb, in_=wot)
            nc.sync.dma_start(out=ident_sb, in_=ident)
            ident_dve = singles.tile([P, P], f32)
            nc.vector.tensor_copy(ident_dve, ident_sb)
            nc.sync.dma_start(out=brow_sb, in_=brow)
            nc.sync.dma_start(out=ones_sb, in_=onesrow)

            xs_t = xs.rearrange("(t c p) f -> t c p f", c=NCHUNK, p=P)
            out_t = out.rearrange("(t c p) o -> t c p o", c=NCHUNK, p=P)

            for it in range(n_tiles):
                # ---- load: [128, 4, 384], chunk c = atoms it*512+c*128 ... +127
                x_in = xin_pool.tile([P, NCHUNK, F], f32)
                nc.sync.dma_start(
                    out=x_in,
                    in_=xs_t[it].rearrange("c p f -> p c f"),
                )
                x_uj = x_in.rearrange("p c (u j) -> p c u j", j=EDIM)

                # ---- per-atom sum of squares -> stats[:, c]
                stats = stats_pool.tile([P, NCHUNK], f32)
                ms_eng = nc.gpsimd if ms_engine == "gpsimd" else nc.vector
                for c in range(NCHUNK):
                    sq_junk = sq_pool.tile([P, F], f32, tag="sq")
                    ms_eng.scalar_tensor_tensor(
                        out=sq_junk,
                        in0=x_in[:, c],
                        scalar=1.0,
                        in1=x_in[:, c],
                        op0=Alu.mult,
                        op1=Alu.mult,
                        accum_out=stats[:, c : c + 1],
                    )

                # ---- s = 1/sqrt(ms + eps) (per atom)
                snorm = stats_pool.tile([P, NCHUNK], f32)
                nc.scalar.activation(
                    out=snorm, in_=stats, func=Act.Sqrt, scale=1.0 / F, bias=EPS
                )
                s_rec = stats_pool.tile([P, NCHUNK], f32)
                nc.vector.reciprocal(out=s_rec, in_=snorm)

                # ---- transposes with fused normalization: xT[j][u, t-in-tile]
                xt_ps = [
                    psum_pool.tile([P, TILE_ATOMS], f32, tag="ps", name=f"xtps{j}")
                    for j in range(EDIM)
                ]
                for c in range(NCHUNK):
                    diag_c = diag_pool.tile([P, P], f32, tag="diag")
                    nc.scalar.activation(
                        out=diag_c,
                        in_=ident_dve,
                        func=Act.Copy,
                        scale=s_rec[:, c : c + 1],
                    )
                    for j in range(EDIM):
                        nc.tensor.matmul(
                            xt_ps[j][:, c * P : (c + 1) * P],
                            x_uj[:, c, :, j],
                            diag_c,
                            start=True,
                            stop=True,
                        )
                xt_sb = []
                for j in range(EDIM):
                    t = xt_pool.tile([P, TILE_ATOMS], f32, tag="xt", name=f"xt{j}")
                    nc.scalar.copy(t, xt_ps[j])
                    xt_sb.append(t)

                # ---- y_k = w1' @ xh_k   (PSUM)
                y_ps = [
                    psum_pool.tile([P, TILE_ATOMS], f32, tag="ps", name=f"yps{k}")
                    for k in range(EDIM)
                ]
                for k in range(EDIM):
                    nc.tensor.matmul(
                        y_ps[k], w1t_sb, xt_sb[k], start=True, stop=True
                    )

                # ---- B products for the cross terms
                bprod = {}
                for (a, b) in PLUS + MINUS:
                    t = b_pool.tile([P, TILE_ATOMS], f32, tag="bp", name=f"bp{a}{b}")
                    nc.vector.tensor_mul(t, xt_sb[a], y_ps[b])
                    bprod[(a, b)] = t

                # ---- z_i = w2' @ B_plus[i] - w2' @ B_minus[i]  (PSUM accumulate)
                z_ps = [
                    psum_pool.tile([P, TILE_ATOMS], f32, tag="ps", name=f"zps{i}")
                    for i in range(EDIM)
                ]
                for i in range(EDIM):
                    nc.tensor.matmul(
                        z_ps[i], w2pt_sb, bprod[PLUS[i]], start=True, stop=False
                    )
                    nc.tensor.matmul(
                        z_ps[i], w2mt_sb, bprod[MINUS[i]], start=False, stop=True
                    )

                # ---- chi = sum_i xh_i * z_i
                cprod = []
                for i in range(EDIM):
                    t = c_pool.tile([P, TILE_ATOMS], f32, tag="cp", name=f"cp{i}")
                    nc.vector.tensor_mul(t, xt_sb[i], z_ps[i])
                    cprod.append(t)
                chi01 = chi_pool.tile([P, TILE_ATOMS], f32, tag="chi")
                nc.gpsimd.tensor_add(chi01, cprod[0], cprod[1])
                chi = chi_pool.tile([P, TILE_ATOMS], f32, tag="chi")
                nc.gpsimd.tensor_add(chi, chi01, cprod[2])

                # ---- out chunks: psum = ones x b (bias) + chi_chunk.T @ Wo'
                out_sb = out_pool.tile([P, NCHUNK, OUT], f32)
                for c in range(NCHUNK):
                    o_ps = psum_pool.tile([P, OUT], f32, tag="ps")
                    nc.tensor.matmul(o_ps, ones_sb, brow_sb, start=True, stop=False)
                    nc.tensor.matmul(
                        o_ps,
                        chi[:, c * P : (c + 1) * P],
                        wot_sb,
                        start=False,
                        stop=True,
                    )
                    nc.scalar.copy(out_sb[:, c], o_ps)
                nc.sync.dma_start(
                    out=out_t[it].rearrange("c p o -> p c o"), in_=out_sb
                )

    nc.finalize()
    return nc


def _host_prep(inputs):
    emb = np.asarray(inputs["atomic_embeddings"], dtype=np.float32)
    g = np.asarray(inputs["rms_g"], dtype=np.float32)
    w1 = np.asarray(inputs["w1"], dtype=np.float32)
    w2 = np.asarray(inputs["w2"], dtype=np.float32)
    W_out = np.asarray(inputs["W_out"], dtype=np.float32)
    b_out = np.asarray(inputs["b_out"], dtype=np.float32)

    xs_full = np.ascontiguousarray(emb[:, INV:])  # [N, 384]
    consts = {
        "w1t": np.ascontiguousarray(C1 * (w1.T * g[:, None])).astype(np.float32),
        "w2pt": np.ascontiguousarray(C2 * (w2.T * g[:, None])).astype(np.float32),
        "w2mt": np.ascontiguousarray(-C2 * (w2.T * g[:, None])).astype(np.float32),
        "wot": np.ascontiguousarray(W_out.T * g[:, None]).astype(np.float32),
        "ident": np.eye(P, dtype=np.float32),
        "brow": b_out.reshape(1, OUT).astype(np.float32),
        "onesrow": np.ones((1, P), dtype=np.float32),
    }
    return xs_full, consts


_NC_CACHE = {}


def _get_nc(n_shard):
    if n_shard not in _NC_CACHE:
        _NC_CACHE[n_shard] = _build_nc(n_shard)
    return _NC_CACHE[n_shard]


def kernel(**inputs) -> np.ndarray:
    from concourse.bass_utils import run_bass_kernel_spmd

    xs_full, consts = _host_prep(inputs)
    n = xs_full.shape[0]
    assert n == N_TOTAL, f"expected {N_TOTAL} atoms, got {n}"

    nc = _get_nc(N_SHARD)
    in_maps = []
    for i in range(N_CORES):
        m = {"xs": xs_full[i * N_SHARD : (i + 1) * N_SHARD]}
        m.update(consts)
        in_maps.append(m)

    res = run_bass_kernel_spmd(nc, in_maps, list(range(N_CORES)))
    return np.concatenate(
        [res.results[i]["out"] for i in range(N_CORES)], axis=0
    ).astype(np.float32)


# revision 12
# speedup vs baseline: 67157.0032x; 67157.0032x over previous
"""Trainium2 Bass kernel for nn_ChiralEmbeddingModel (chiral tensor-product embedding).

Math (per atom n, with x = atomic_embeddings[n, 256:].reshape(128, 3)):
    ms   = mean(x^2)                       (over all 384 components)
    xh   = x / sqrt(ms + eps)              (rms_g folded into the weights)
    y    = w1' @ xh                        (w1'[u,v] = C1 * g[v] * w1[u,v])
    cr_i = eps_ijk xh_j y_k                (cross product per mul-channel)
    z    = w2' @ cr                        (w2'[u,v] = C2 * g[v] * w2[u,v])
    chi  = sum_i xh_i * z_i
    out  = chi @ Wo' + b                   (Wo'[u,o] = g[u] * W_out[o,u])

Device layout: atoms are tiled 512 at a time (4 chunks of 128).  The per-atom
rms scale is fused into the PE transpose (rhs = diag(s) instead of identity),
so everything downstream of the transpose is already normalized.  The cross
product subtractions are folded into the z matmuls via +/-w2' stationaries.
The bias is folded into the output PSUM via a rank-1 (ones x b) matmul.

Sharding: pure data-parallel over the atom axis across 8 NeuronCores; small
weights are replicated.  Host-side prep only slices/reshapes inputs and
transposes/scales the small replicated weight matrices.
"""

import numpy as np

N_TOTAL = 131072
N_CORES = 8
N_SHARD = N_TOTAL // N_CORES  # 16384
INV = 256
MUL = 128
EDIM = 3
F = MUL * EDIM  # 384
OUT = 512
EPS = 1e-6
C1 = (3.0 / 256.0) ** 0.5
C2 = (1.0 / 384.0) ** 0.5
P = 128
TILE_ATOMS = 512
NCHUNK = TILE_ATOMS // P  # 4

# which engine computes the sum-of-squares (scalar_tensor_tensor with accum):
#   "gpsimd" keeps the vector engine free (it is the bottleneck), falls back
#   to "vector" if the Q7 ucode path turns out not to support it.
MS_ENGINE = "vector"
# matmul operand dtype mode: "fp32" (exact, 4 cyc/row) or "fp32r" (reduced
# precision single-pass, 1 cyc/row for N>=256)
MM_DTYPE = "fp32r"


def _build_nc(n_shard: int, ms_engine: str = MS_ENGINE, repeat: int = 1, loop_repeat: int = 1, mm_dtype: str = None):
    import concourse.bass as bass
    import concourse.bacc as bacc
    import concourse.tile as tile
    from concourse import mybir

    f32 = mybir.dt.float32
    Alu = mybir.AluOpType
    Act = mybir.ActivationFunctionType

    if mm_dtype is None:
        mm_dtype = MM_DTYPE
    assert n_shard % TILE_ATOMS == 0
    n_tiles = n_shard // TILE_ATOMS

    nc = bacc.Bacc("TRN2", target_bir_lowering=False, debug=False)

    # Register EPS as a const AP (like the framework's 0.0/1.0) so activation
    # bias=EPS lowers to a dependency-free const read instead of a tracked AP.
    _eps_t = nc.alloc_sbuf_tensor("const-float32-eps", [128, 1], f32)
    nc.gpsimd.memset(_eps_t.ap(), EPS)
    nc.const_aps.aps[(f32, EPS)] = _eps_t.ap()
    nc.all_engine_barrier()

    xs = nc.dram_tensor("xs", [n_shard, F], f32, kind="ExternalInput").ap()
    w1t = nc.dram_tensor("w1t", [MUL, MUL], f32, kind="ExternalInput").ap()
    w2pt = nc.dram_tensor("w2pt", [MUL, MUL], f32, kind="ExternalInput").ap()
    w2mt = nc.dram_tensor("w2mt", [MUL, MUL], f32, kind="ExternalInput").ap()
    wot = nc.dram_tensor("wot", [MUL, OUT], f32, kind="ExternalInput").ap()
    ident = nc.dram_tensor("ident", [P, P], f32, kind="ExternalInput").ap()
    brow = nc.dram_tensor("brow", [1, OUT], f32, kind="ExternalInput").ap()
    onesrow = nc.dram_tensor("onesrow", [1, P], f32, kind="ExternalInput").ap()
    out = nc.dram_tensor("out", [n_shard, OUT], f32, kind="ExternalOutput").ap()

    f32r = mybir.dt.float32r

    def mmcast(ap):
        return ap.bitcast(f32r) if mm_dtype == "fp32r" else ap

    # cross product index pairs: cr_0 = xh1*y2 - xh2*y1, etc.
    PLUS = [(1, 2), (2, 0), (0, 1)]
    MINUS = [(2, 1), (0, 2), (1, 0)]

    with tile.TileContext(nc) as tc:
        with (
            tc.tile_pool(name="singles", bufs=1) as singles,
            tc.tile_pool(name="xin", bufs=3) as xin_pool,
            tc.tile_pool(name="stats", bufs=2) as stats_pool,
            tc.tile_pool(name="sq", bufs=2) as sq_pool,
            tc.tile_pool(name="diag", bufs=8) as diag_pool,
            tc.tile_pool(name="xt", bufs=6) as xt_pool,
            tc.tile_pool(name="bprod", bufs=12) as b_pool,
            tc.tile_pool(name="cprod", bufs=6) as c_pool,
            tc.tile_pool(name="chi", bufs=4) as chi_pool,
            tc.tile_pool(name="outs", bufs=2) as out_pool,
            tc.tile_pool(name="psum", bufs=8, space="PSUM") as psum_pool,
        ):
            # ---- load replicated constants once ----
            w1t_sb = singles.tile([MUL, MUL], f32)
            w2pt_sb = singles.tile([MUL, MUL], f32)
            w2mt_sb = singles.tile([MUL, MUL], f32)
            wot_sb = singles.tile([MUL, OUT], f32)
            ident_sb = singles.tile([P, P], f32)
            brow_sb = singles.tile([1, OUT], f32)
            ones_sb = singles.tile([1, P], f32)
            nc.sync.dma_start(out=w1t_sb, in_=w1t)
            nc.sync.dma_start(out=w2pt_sb, in_=w2pt)
            nc.sync.dma_start(out=w2mt_sb, in_=w2mt)
            nc.sync.dma_start(out=wot_sb, in_=wot)
            nc.sync.dma_start(out=ident_sb, in_=ident)
            ident_dve = singles.tile([P, P], f32)
            nc.vector.tensor_copy(ident_dve, ident_sb)
            nc.sync.dma_start(out=brow_sb, in_=brow)
            nc.sync.dma_start(out=ones_sb, in_=onesrow)

            xs_t = xs.rearrange("(t c p) f -> t c p f", c=NCHUNK, p=P)
            out_t = out.rearrange("(t c p) o -> t c p o", c=NCHUNK, p=P)

            import contextlib

            loop_cm = (
                tc.For_i(0, loop_repeat, 1)
                if loop_repeat > 1
                else contextlib.nullcontext()
            )
            with loop_cm:
             for _rep in range(repeat):
              for it in range(n_tiles):
                # ---- load: [128, 4, 384], chunk c = atoms it*512+c*128 ... +127
                x_in = xin_pool.tile([P, NCHUNK, F], f32)
                nc.sync.dma_start(
                    out=x_in,
                    in_=xs_t[it].rearrange("c p f -> p c f"),
                )
                x_uj = x_in.rearrange("p c (u j) -> p c u j", j=EDIM)

                # ---- per-atom sum of squares -> stats[:, c]
                stats = stats_pool.tile([P, NCHUNK], f32)
                ms_eng = nc.gpsimd if ms_engine == "gpsimd" else nc.vector
                for c in range(NCHUNK):
                    sq_junk = sq_pool.tile([P, F], f32, tag="sq")
                    ms_eng.scalar_tensor_tensor(
                        out=sq_junk,
                        in0=x_in[:, c],
                        scalar=1.0,
                        in1=x_in[:, c],
                        op0=Alu.mult,
                        op1=Alu.mult,
                        accum_out=stats[:, c : c + 1],
                    )

                # ---- s = 1/sqrt(ms + eps) (per atom)
                snorm = stats_pool.tile([P, NCHUNK], f32)
                nc.scalar.activation(
                    out=snorm, in_=stats, func=Act.Sqrt, scale=1.0 / F, bias=EPS
                )
                s_rec = stats_pool.tile([P, NCHUNK], f32)
                nc.vector.reciprocal(out=s_rec, in_=snorm)

                # ---- transposes with fused normalization: xT[j][u, t-in-tile]
                xt_ps = [
                    psum_pool.tile([P, TILE_ATOMS], f32, tag="ps", name=f"xtps{j}")
                    for j in range(EDIM)
                ]
                for c in range(NCHUNK):
                    diag_c = diag_pool.tile([P, P], f32, tag="diag")
                    nc.scalar.activation(
                        out=diag_c,
                        in_=ident_dve,
                        func=Act.Copy,
                        scale=s_rec[:, c : c + 1],
                    )
                    for j in range(EDIM):
                        nc.tensor.matmul(
                            xt_ps[j][:, c * P : (c + 1) * P],
                            mmcast(x_uj[:, c, :, j]),
                            mmcast(diag_c),
                            start=True,
                            stop=True,
                        )
                xt_sb = []
                for j in range(EDIM):
                    t = xt_pool.tile([P, TILE_ATOMS], f32, tag="xt", name=f"xt{j}")
                    nc.scalar.copy(t, xt_ps[j])
                    xt_sb.append(t)

                # ---- y_k = w1' @ xh_k   (PSUM)
                y_ps = [
                    psum_pool.tile([P, TILE_ATOMS], f32, tag="ps", name=f"yps{k}")
                    for k in range(EDIM)
                ]
                for k in range(EDIM):
                    nc.tensor.matmul(
                        y_ps[k], mmcast(w1t_sb), mmcast(xt_sb[k]),
                        start=True, stop=True,
                    )

                # ---- B products for the cross terms
                bprod = {}
                for (a, b) in PLUS + MINUS:
                    t = b_pool.tile([P, TILE_ATOMS], f32, tag="bp", name=f"bp{a}{b}")
                    nc.vector.tensor_mul(t, xt_sb[a], y_ps[b])
                    bprod[(a, b)] = t

                # ---- z_i = w2' @ B_plus[i] - w2' @ B_minus[i]  (PSUM accumulate)
                z_ps = [
                    psum_pool.tile([P, TILE_ATOMS], f32, tag="ps", name=f"zps{i}")
                    for i in range(EDIM)
                ]
                for i in range(EDIM):
                    nc.tensor.matmul(
                        z_ps[i], mmcast(w2pt_sb), mmcast(bprod[PLUS[i]]),
                        start=True, stop=False,
                    )
                    nc.tensor.matmul(
                        z_ps[i], mmcast(w2mt_sb), mmcast(bprod[MINUS[i]]),
                        start=False, stop=True,
                    )

                # ---- chi = sum_i xh_i * z_i
                cprod = []
                for i in range(EDIM):
                    t = c_pool.tile([P, TILE_ATOMS], f32, tag="cp", name=f"cp{i}")
                    nc.vector.tensor_mul(t, xt_sb[i], z_ps[i])
                    cprod.append(t)
                chi01 = chi_pool.tile([P, TILE_ATOMS], f32, tag="chi")
                nc.gpsimd.tensor_add(chi01, cprod[0], cprod[1])
                chi = chi_pool.tile([P, TILE_ATOMS], f32, tag="chi")
                nc.gpsimd.tensor_add(chi, chi01, cprod[2])

                # ---- out chunks: psum = ones x b (bias) + chi_chunk.T @ Wo'
                out_sb = out_pool.tile([P, NCHUNK, OUT], f32)
                for c in range(NCHUNK):
                    o_ps = psum_pool.tile([P, OUT], f32, tag="ps")
                    nc.tensor.matmul(
                        o_ps, mmcast(ones_sb), mmcast(brow_sb),
                        start=True, stop=False,
                    )
                    nc.tensor.matmul(
                        o_ps,
                        mmcast(chi[:, c * P : (c + 1) * P]),
                        mmcast(wot_sb),
                        start=False,
                        stop=True,
                    )
                    nc.scalar.copy(out_sb[:, c], o_ps)
                nc.sync.dma_start(
                    out=out_t[it].rearrange("c p o -> p c o"), in_=out_sb
                )

    nc.finalize()
    return nc


def _host_prep(inputs):
    emb = np.asarray(inputs["atomic_embeddings"], dtype=np.float32)
    g = np.asarray(inputs["rms_g"], dtype=np.float32)
    w1 = np.asarray(inputs["w1"], dtype=np.float32)
    w2 = np.asarray(inputs["w2"], dtype=np.float32)
    W_out = np.asarray(inputs["W_out"], dtype=np.float32)
    b_out = np.asarray(inputs["b_out"], dtype=np.float32)

    xs_full = np.ascontiguousarray(emb[:, INV:])  # [N, 384]
    consts = {
        "w1t": np.ascontiguousarray(C1 * (w1.T * g[:, None])).astype(np.float32),
        "w2pt": np.ascontiguousarray(C2 * (w2.T * g[:, None])).astype(np.float32),
        "w2mt": np.ascontiguousarray(-C2 * (w2.T * g[:, None])).astype(np.float32),
        "wot": np.ascontiguousarray(W_out.T * g[:, None]).astype(np.float32),
        "ident": np.eye(P, dtype=np.float32),
        "brow": b_out.reshape(1, OUT).astype(np.float32),
        "onesrow": np.ones((1, P), dtype=np.float32),
    }
    return xs_full, consts


_NC_CACHE = {}


def _get_nc(n_shard):
    if n_shard not in _NC_CACHE:
        _NC_CACHE[n_shard] = _build_nc(n_shard)
    return _NC_CACHE[n_shard]


def kernel(**inputs) -> np.ndarray:
    from concourse.bass_utils import run_bass_kernel_spmd

    xs_full, consts = _host_prep(inputs)
    n = xs_full.shape[0]
    assert n == N_TOTAL, f"expected {N_TOTAL} atoms, got {n}"

    nc = _get_nc(N_SHARD)
    in_maps = []
    for i in range(N_CORES):
        m = {"xs": xs_full[i * N_SHARD : (i + 1) * N_SHARD]}
        m.update(consts)
        in_maps.append(m)

    res = run_bass_kernel_spmd(nc, in_maps, list(range(N_CORES)))
    return np.concatenate(
        [res.results[i]["out"] for i in range(N_CORES)], axis=0
    ).astype(np.float32)
